# revision 48
# baseline (speedup 1.0000x reference)
"""Trainium2 Bass kernel for nn_Classifier_8461085573484 (2-layer GCN classifier).

Math: with x [N,1] and b1=0, both GCN layers collapse to scalar per-node values:
  degp1_d = indeg(d)+1;  dinv = 1/sqrt(degp1);  u = x*dinv
  S_d  = sum_{e->d} u[src];   y = dinv^2 * (S + x*dinv)   (y = dinv * y_true)
  SP_d = sum_{e->d} relu(y[src]);  SN_d = sum_{e->d} min(y[src],0)
  alpha = dinv*(SP + relu(y));     beta = dinv*(relu(-y) - SN)
  z     = relu(alpha a^T + beta b^T + b2), a = relu(W1)@W2, b = relu(-W1)@W2
  logits = mean(z) @ Wl + bl -> log_softmax.

Sharding (8 NeuronCores): nodes sorted by a per-pass key (A: degree,
B: max(pos,neg) edge count), dealt round-robin to cores; local rank l ->
(q=l//128, lane=l%128).  Columns grouped into cap REGIONS (DP-chosen), each
node owning a cap-wide window in its lane's row, so every segment-sum is a
dense strided tensor_reduce.  Degree sorting makes per-region caps tight
(~72/80 slots/node vs max-degree 105/121 uniform caps).
Pass A (kA): per chunk, xs fp16 + degp1 uint8 slots stream in; Scalar ACT
computes rsqrt(degp1[src]), Vector multiplies, GpSimd folds window halves
(once or twice), Vector windowed-reduces -> S -> y, dinv grids.
Pass B (k5): y[src] fp8 slots, sign-split into EQUAL pos/neg half-windows
(cap per region = max(pos,neg)) so ONE 4D reduce [p,q,2,c] yields SP and SN
interleaved; GpSimd pre-folds all large chunks.  Head: per-feature
w_f = main + (small/big)*other (16 Vector STTs), two sign-grouped Relus
(Scalar), two windowed reduces; the per-feature |big| rescale joins the
O(1) host head.  Pad nodes contribute exactly relu(b2) each, also
corrected in the host head.
Host only counts/permutes; all O(N)/O(E) float math runs on device.
"""
import contextlib
import ctypes
import sys
import types

import numpy as np
import ml_dtypes

from concourse import bacc, bass, mybir
import concourse.tile as tile
from concourse import bass_utils

P = 128
Q = 98
NSH = P * Q            # 12544 nodes per NC shard
NC = 8
NPAD = NSH * NC        # 100352
N = 100000
F32 = mybir.dt.float32
F16 = mybir.dt.float16
F8 = mybir.dt.float8e4
U8 = mybir.dt.uint8
E4NP = ml_dtypes.float8_e4m3fn


def _install_ntff_shim():
    """Provide antenv.axon_hooks so run_bass_kernel_spmd(trace=True) works."""
    if "antenv.axon_hooks" in sys.modules:
        return
    import antenv

    _hook = None
    try:
        lib = ctypes.CDLL("/opt/axon/libaxon_pjrt.so")
        if hasattr(lib, "axon_start_nrt_profile"):
            lib.axon_start_nrt_profile.argtypes = [
                ctypes.POINTER(ctypes.c_int64), ctypes.c_size_t]
            lib.axon_start_nrt_profile.restype = ctypes.c_int64
            lib.axon_stop_nrt_profile.argtypes = [ctypes.c_char_p]
            lib.axon_stop_nrt_profile.restype = ctypes.c_int64

            @contextlib.contextmanager
            def _hook_impl(output_dir, device_ids):
                import jax
                jax.devices()
                if device_ids:
                    ids = (ctypes.c_int64 * len(device_ids))(*device_ids)
                    rc = lib.axon_start_nrt_profile(ids, len(device_ids))
                else:
                    rc = lib.axon_start_nrt_profile(None, 0)
                if rc != 0:
                    raise RuntimeError(f"axon_start_nrt_profile rc={rc}")
                try:
                    yield
                finally:
                    n = lib.axon_stop_nrt_profile(str(output_dir).encode())
                    if n < 0:
                        raise RuntimeError(f"axon_stop_nrt_profile rc={n}")

            _hook = _hook_impl
    except OSError:
        pass

    mod = types.ModuleType("antenv.axon_hooks")
    mod._hook = _hook
    mod.get_axon_ntff_profile_hook = lambda: mod._hook

    def set_axon_ntff_profile_hook(h):
        mod._hook = h

    mod.set_axon_ntff_profile_hook = set_axon_ntff_profile_hook
    sys.modules["antenv.axon_hooks"] = mod
    antenv.axon_hooks = mod


_install_ntff_shim()


# ---------------- host layout (counting / permutation only) ----------------

def _dp_regions(qmax, align, penalty):
    """Partition q-columns into cap regions minimizing slots + region cost."""
    INF = float("inf")
    best = [INF] * (Q + 1)
    best[0] = 0.0
    back = [0] * (Q + 1)
    for j in range(1, Q + 1):
        cm = 0
        for i in range(j - 1, -1, -1):
            cm = max(cm, int(qmax[i]))
            c8 = ((cm + align - 1) // align) * align
            v = best[i] + (j - i) * c8 + penalty
            if v < best[j]:
                best[j] = v
                back[j] = i
    regs = []
    j = Q
    while j > 0:
        i = back[j]
        cm = int(qmax[i:j].max())
        regs.append((i, j - i, ((cm + align - 1) // align) * align))
        j = i
    regs.reverse()
    return regs


class Layout:
    """Node->(core,lane,q) map from a sort key + per-region window caps."""

    def __init__(self, key, align, penalty, wfactor=1):
        order = np.argsort(-key, kind="stable")  # high key first
        rank = np.empty(N, np.int64)
        rank[order] = np.arange(N)
        self.order = order
        self.rank = rank              # node -> global rank
        self.core = rank % NC
        loc = rank // NC
        self.lane = loc % P
        self.q = loc // P
        keyp = np.zeros(NPAD, np.int64)
        keyp[:N] = key[order]
        q_of = (np.arange(NPAD) // NC) // P
        qmax = np.zeros(Q, np.int64)
        np.maximum.at(qmax, q_of, keyp)
        self.regions = _dp_regions(qmax, align, penalty)
        # per-q window cap and column base (in units of wfactor*cap)
        self.capq = np.zeros(Q, np.int64)
        self.qbase = np.zeros(Q, np.int64)
        off = 0
        for (qa, qn, cap) in self.regions:
            self.capq[qa:qa + qn] = cap
            for q in range(qa, qa + qn):
                self.qbase[q] = off
                off += wfactor * cap
        self.rowlen = int(off)        # slot columns per lane

    def grid_scatter(self, vals, fill, dtype):
        """Per-node vals -> [NC, P, Q] grids in this layout."""
        g = np.full((NC, P, Q), fill, dtype)
        g[self.core, self.lane, self.q] = vals
        return g

    def grid_gather(self, grids):
        """[NC, P, Q] device grids -> per-node flat values."""
        return np.ascontiguousarray(grids[self.core, self.lane, self.q])


def _within_dst(dst, sub=None):
    """Occurrence counter of each edge within its destination (optionally
    within a sub-key such as sign)."""
    key = dst if sub is None else dst * 2 + sub
    order = np.argsort(key, kind="stable")
    k_sorted = key[order]
    e = dst.shape[0]
    cnt = np.bincount(k_sorted)
    starts = np.zeros(cnt.shape[0], np.int64)
    starts[1:] = np.cumsum(cnt)[:-1]
    within = np.empty(e, np.int64)
    within[order] = np.arange(e) - starts[k_sorted]
    return within


# ---------------- device kernel A ----------------

def build_kA(chunks, sa16, sa8, sa_len):
    """Per-slot u = x[src]*rsqrt(degp1[src]) (Scalar ars; mult on Vector for
    fp16 chunks or GpSimd for fp8 chunks), GpSimd half-window folds, Vector
    windowed reduce -> S; grid -> y, dinv.
    chunks: (mode, ds_off, xs_off, q0, qn, cap, eng)."""
    nc = bacc.Bacc("TRN2", target_bir_lowering=False, debug=False)
    xs = nc.dram_tensor("xs", [P, max(sa16, 1)], F16, kind="ExternalInput")
    xs8 = nc.dram_tensor("xs8", [P, max(sa8, 1)], F8, kind="ExternalInput")
    ds = nc.dram_tensor("ds", [P, sa_len], U8, kind="ExternalInput")
    dgp = nc.dram_tensor("degp1", [P, Q], U8, kind="ExternalInput")
    xg = nc.dram_tensor("xg", [P, Q], F32, kind="ExternalInput")
    y_o = nc.dram_tensor("yg", [P, Q], F32, kind="ExternalOutput")
    dinv_o = nc.dram_tensor("dinv", [P, Q], F16, kind="ExternalOutput")
    cmax = max(qn * cap for c in chunks for qn, cap in [(c[4], c[5])])
    with tile.TileContext(nc) as tc:
        with tc.tile_pool(name="sb", bufs=1) as pool, \
             tc.tile_pool(name="blk", bufs=7) as bpool:
            S_sb = pool.tile([P, Q], F32, tag="S")
            for ci, (mode, doff, xoff, q0, qn, cap, eng) in enumerate(chunks):
                n = qn * cap
                isv = eng == "V"
                xs_b = bpool.tile([P, cmax], F16 if isv else F8,
                                  tag=f"xs{eng}")
                ds_b = bpool.tile([P, cmax], U8, tag="ds")
                # ds feeds Scalar (the pipeline pacer): issue it first
                nc.sync.dma_start(ds_b[:, 0:n], ds.ap()[:, doff:doff + n])
                nc.sync.dma_start(xs_b[:, 0:n],
                                  (xs if isv else xs8).ap()[:, xoff:xoff + n])
                ars_b = bpool.tile([P, cmax], F16, tag="ars")
                nc.scalar.activation(
                    out=ars_b[:, 0:n], in_=ds_b[:, 0:n],
                    func=mybir.ActivationFunctionType.Abs_reciprocal_sqrt)
                v_b = bpool.tile([P, cmax], F16, tag="v")
                me = nc.vector if isv else nc.gpsimd
                me.tensor_tensor(out=v_b[:, 0:n], in0=xs_b[:, 0:n],
                                 in1=ars_b[:, 0:n], op=mybir.AluOpType.mult)
                red, rc = v_b, cap
                if mode in ("F", "FF"):
                    c2 = cap // 2
                    v3 = v_b[:, 0:n].rearrange("p (q c) -> p q c", c=cap)
                    f_b = bpool.tile([P, cmax // 2], F16, tag="f1")
                    nc.gpsimd.tensor_tensor(
                        out=f_b[:, 0:n // 2].rearrange("p (q c) -> p q c",
                                                       c=c2),
                        in0=v3[:, :, 0:c2], in1=v3[:, :, c2:cap],
                        op=mybir.AluOpType.add)
                    red, rc = f_b, c2
                    if mode == "FF":
                        c4 = c2 // 2
                        f3 = f_b[:, 0:n // 2].rearrange("p (q c) -> p q c",
                                                        c=c2)
                        g_b = bpool.tile([P, cmax // 4], F16, tag="f2")
                        nc.gpsimd.tensor_tensor(
                            out=g_b[:, 0:n // 4].rearrange(
                                "p (q c) -> p q c", c=c4),
                            in0=f3[:, :, 0:c4], in1=f3[:, :, c4:c2],
                            op=mybir.AluOpType.add)
                        red, rc = g_b, c4
                nc.vector.tensor_reduce(
                    out=S_sb[:, q0:q0 + qn],
                    in_=red[:, 0:qn * rc].rearrange("p (q c) -> p q c", c=rc),
                    axis=mybir.AxisListType.X, op=mybir.AluOpType.add)
            dgp_sb = pool.tile([P, Q], U8, tag="dgp")
            xg_sb = pool.tile([P, Q], F32, tag="xg")
            nc.sync.dma_start(dgp_sb[:], dgp.ap())
            nc.sync.dma_start(xg_sb[:], xg.ap())
            dinv_sb = pool.tile([P, Q], F16, tag="dinv")
            ug_sb = pool.tile([P, Q], F32, tag="ug")
            nc.scalar.activation(
                out=dinv_sb[:], in_=dgp_sb[:],
                func=mybir.ActivationFunctionType.Abs_reciprocal_sqrt)
            nc.vector.tensor_tensor(out=ug_sb[:], in0=xg_sb[:], in1=dinv_sb[:],
                                    op=mybir.AluOpType.mult)
            t = pool.tile([P, Q], F32, tag="t")
            d2 = pool.tile([P, Q], F32, tag="d2")
            nc.vector.tensor_tensor(out=d2[:], in0=dinv_sb[:], in1=dinv_sb[:],
                                    op=mybir.AluOpType.mult)
            nc.vector.tensor_tensor(out=t[:], in0=S_sb[:], in1=ug_sb[:],
                                    op=mybir.AluOpType.add)
            nc.vector.tensor_tensor(out=t[:], in0=t[:], in1=d2[:],
                                    op=mybir.AluOpType.mult)
            nc.sync.dma_start(y_o.ap(), t[:])
            nc.sync.dma_start(dinv_o.ap(), dinv_sb[:])
    nc.compile()
    return nc


# ---------------- device kernel B ----------------

def _w_plan(a_vec, b_vec, b2_vec):
    """Per-feature rewrite z_f = relu(a_f*alpha + b_f*beta + b2_f)
    = s_f * relu(sig_f * (main + r_f*other) + b2_f/|..|):  pick the larger
    coefficient as the 'main' term so |r_f| <= 1.  Returns per-feature
    (use_alpha_main, r, sigma, scale) with features reordered so that
    sigma=+1 features come first (two grouped relus)."""
    plan = []
    for f in range(16):
        a, b, c = float(a_vec[f]), float(b_vec[f]), float(b2_vec[f])
        if abs(a) >= abs(b):
            main_alpha, big, r = True, a, (b / a if a != 0 else 0.0)
        else:
            main_alpha, big, r = False, b, a / b
        if big == 0.0:
            # z_f = relu(b2_f): constant per node; handled on host
            plan.append((True, 0.0, 1.0, 0.0, c, f))
            continue
        sig = 1.0 if big > 0 else -1.0
        plan.append((main_alpha, r, sig, abs(big), c / abs(big), f))
    plan.sort(key=lambda t: -t[2])  # sigma=+1 first
    return plan


def build_kB(chunks, sb_len, a_vec, b_vec, b2_vec):
    """Sign-split equal-cap windows: one 4D reduce per chunk -> SP|SN.
    GpSimd pre-folds chunks flagged 'F'.  Head: alpha/beta, per-feature
    w_f = main + r_f*other (STT), grouped relus, one windowed reduce,
    per-feature rescale."""
    nc = bacc.Bacc("TRN2", target_bir_lowering=False, debug=False)
    ys = nc.dram_tensor("ys", [P, sb_len], F8, kind="ExternalInput")
    dinv = nc.dram_tensor("dinvg", [P, Q], F16, kind="ExternalInput")
    yg = nc.dram_tensor("yg", [P, Q], F32, kind="ExternalInput")

    acc_o = nc.dram_tensor("acc", [P, 16], F32, kind="ExternalOutput")
    plan = _w_plan(a_vec, b_vec, b2_vec)
    use_b2 = bool(np.any(b2_vec != 0))
    if use_b2:
        for (_, _, _, _, bias, _) in plan:
            if (mybir.dt.float32, bias) not in nc.const_aps.aps:
                t = nc.alloc_sbuf_tensor(f"const-b2-{bias}", [128, 1], F32)
                nc.gpsimd.memset(t.ap(), bias)
                nc.const_aps.aps[(mybir.dt.float32, bias)] = t.ap()
    cmax = max(qn * 2 * cap for _, _, _, qn, cap in chunks)
    with tile.TileContext(nc) as tc:
        with tc.tile_pool(name="sb", bufs=1) as pool, \
             tc.tile_pool(name="blk", bufs=7) as bpool:
            # SPN layout: [:, 0:Q] = SP, [:, Q:2Q] = SN (via strided out AP)
            SPN = pool.tile([P, 2 * Q], F32, tag="SPN")
            for ci, (fold, off, q0, qn, cap) in enumerate(chunks):
                n = qn * 2 * cap
                ys_b = bpool.tile([P, cmax], F8, tag="ys")
                nc.sync.dma_start(ys_b[:, 0:n], ys.ap()[:, off:off + n])
                out_ap = SPN[:].rearrange("p (s Qq) -> p Qq s",
                                          s=2)[:, q0:q0 + qn, :]
                if fold in ("F", "FF"):
                    c2 = cap // 2
                    v4 = ys_b[:, 0:n].rearrange("p (q s c) -> p q s c", s=2,
                                                c=cap)
                    f_b = bpool.tile([P, cmax // 2], F16, tag="f")
                    f4 = f_b[:, 0:n // 2].rearrange("p (q s c) -> p q s c",
                                                    s=2, c=c2)
                    nc.gpsimd.tensor_tensor(out=f4, in0=v4[:, :, :, 0:c2],
                                            in1=v4[:, :, :, c2:cap],
                                            op=mybir.AluOpType.add)
                    if fold == "FF":
                        c4 = c2 // 2
                        g_b = bpool.tile([P, cmax // 4], F16, tag="f2b")
                        g4 = g_b[:, 0:n // 4].rearrange(
                            "p (q s c) -> p q s c", s=2, c=c4)
                        nc.gpsimd.tensor_tensor(out=g4,
                                                in0=f4[:, :, :, 0:c4],
                                                in1=f4[:, :, :, c4:c2],
                                                op=mybir.AluOpType.add)
                        f4 = g4
                    nc.vector.tensor_reduce(out=out_ap, in_=f4,
                                            axis=mybir.AxisListType.X,
                                            op=mybir.AluOpType.add)
                else:
                    nc.vector.tensor_reduce(
                        out=out_ap,
                        in_=ys_b[:, 0:n].rearrange("p (q s c) -> p q s c",
                                                   s=2, c=cap),
                        axis=mybir.AxisListType.X, op=mybir.AluOpType.add)
            dinv_sb = pool.tile([P, Q], F16, tag="dinv")
            y_sb = pool.tile([P, Q], F32, tag="yg")
            nc.scalar.dma_start(dinv_sb[:], dinv.ap())
            nc.scalar.dma_start(y_sb[:], yg.ap())
            # alpha = dinv*(SP + relu(y)); beta = dinv*(relu(-y) - SN)
            ry = pool.tile([P, Q], F16, tag="ry")
            rmy = pool.tile([P, Q], F16, tag="rmy")
            nc.scalar.activation(out=ry[:], in_=y_sb[:],
                                 func=mybir.ActivationFunctionType.Relu)
            nc.scalar.activation(out=rmy[:], in_=y_sb[:],
                                 func=mybir.ActivationFunctionType.Relu,
                                 scale=-1.0)
            alpha = pool.tile([P, Q], F16, tag="alpha")
            beta = pool.tile([P, Q], F16, tag="beta")
            nc.vector.tensor_tensor(out=alpha[:], in0=SPN[:, 0:Q], in1=ry[:],
                                    op=mybir.AluOpType.add)
            nc.vector.tensor_tensor(out=alpha[:], in0=alpha[:],
                                    in1=dinv_sb[:], op=mybir.AluOpType.mult)
            nc.vector.tensor_tensor(out=beta[:], in0=rmy[:], in1=SPN[:, Q:2 * Q],
                                    op=mybir.AluOpType.subtract)
            nc.vector.tensor_tensor(out=beta[:], in0=beta[:], in1=dinv_sb[:],
                                    op=mybir.AluOpType.mult)
            # per sign group: w_f = main + r_f*other (STT), relu, reduce --
            # grouped so relu/reduce of the first group overlap later STTs
            npos = sum(1 for t in plan if t[2] > 0)
            w = pool.tile([P, 16 * Q], F16, tag="w")
            zr = pool.tile([P, 16 * Q], F16, tag="zr")
            acc_sb = pool.tile([P, 16], F32, tag="accp")
            groups = [g for g in [(0, npos, 1.0), (npos, 16, -1.0)]
                      if g[1] > g[0]]
            for (j0, j1, sig) in groups:
                for j in range(j0, j1):
                    amain, r, _, s, bias, f = plan[j]
                    main = alpha if amain else beta
                    oth = beta if amain else alpha
                    nc.vector.scalar_tensor_tensor(
                        out=w[:, j * Q:(j + 1) * Q], in0=oth[:], scalar=r,
                        in1=main[:], op0=mybir.AluOpType.mult,
                        op1=mybir.AluOpType.add)
                if not use_b2:
                    nc.scalar.activation(
                        out=zr[:, j0 * Q:j1 * Q], in_=w[:, j0 * Q:j1 * Q],
                        func=mybir.ActivationFunctionType.Relu, scale=sig)
                else:
                    for j in range(j0, j1):
                        nc.scalar.activation(
                            out=zr[:, j * Q:(j + 1) * Q],
                            in_=w[:, j * Q:(j + 1) * Q],
                            func=mybir.ActivationFunctionType.Relu, scale=sig,
                            bias=plan[j][4])
                nc.vector.tensor_reduce(
                    out=acc_sb[:, j0:j1],
                    in_=zr[:, j0 * Q:j1 * Q].rearrange("p (f q) -> p f q",
                                                       f=j1 - j0),
                    axis=mybir.AxisListType.X, op=mybir.AluOpType.add)
            nc.sync.dma_start(acc_o.ap(), acc_sb[:])
    nc.compile()
    return nc


# ---------------- chunk planning ----------------

def _split_chunks(regions, target, first_small=False, last_small=True):
    """Split cap regions into pipeline chunks of ~target lane-elems."""
    chunks = []
    for (qa, qn, cap) in regions:
        npieces = max(1, round(qn * cap / target))
        npieces = min(npieces, qn)
        base = qn // npieces
        rem = qn % npieces
        q = qa
        for i in range(npieces):
            n = base + (1 if i < rem else 0)
            chunks.append((q, n, cap))
            q += n
    if first_small and chunks:
        # graduated lead-in: 256, 512 lane-elem chunks while DMA ramps up
        for want in (512, 256):
            q0, qn, cap = chunks[0]
            qs = max(1, round(want / cap))
            if qn > qs:
                chunks[0] = (q0 + qs, qn - qs, cap)
                chunks.insert(0, (q0, qs, cap))
        if last_small:
            q0, qn, cap = chunks[-1]
            qs = max(1, round(256 / cap))
            if qn > qs:
                chunks[-1] = (q0, qn - qs, cap)
                chunks.append((q0 + qn - qs, qs, cap))
    return chunks


# ---------------- pipeline ----------------

def run_pipeline(inputs, trace=False):
    x = np.asarray(inputs["x"]).reshape(-1).astype(np.float32)
    ei = np.asarray(inputs["edge_index"])
    src = ei[0].astype(np.int64)
    dst = ei[1].astype(np.int64)
    W1 = np.asarray(inputs["W1"]).astype(np.float64)[0]
    W2 = np.asarray(inputs["W2"]).astype(np.float64)
    b2 = np.asarray(inputs["b2"]).astype(np.float64)
    Wl = np.asarray(inputs["Wl"]).astype(np.float64)
    bl = np.asarray(inputs["bl"]).astype(np.float64)
    a_vec = np.maximum(W1, 0) @ W2
    b_vec = np.maximum(-W1, 0) @ W2

    deg = np.bincount(dst, minlength=N)
    degp1 = (deg + 1).astype(np.int64)

    phase_ns = {}

    def run(nc, in_maps, name):
        res = bass_utils.run_bass_kernel_spmd(
            nc, in_maps, core_ids=list(range(NC)), trace=trace)
        phase_ns[name] = res.exec_time_ns
        return res.results

    # ---- pass A layout & routing ----
    LA = Layout(deg, align=8, penalty=300)
    chunksA = _split_chunks(LA.regions, target=900, first_small=True,
                            last_small=False)
    # fold plan: GpSimd folds every chunk once; second fold on later chunks
    # while GpSimd has slack (rates: V mult 1.5, V red 0.8, G fold 0.9)
    tot = sum(qn * cap for _, qn, cap in chunksA)
    g_load = tot / 0.9
    v_load = tot / 1.5 + (tot / 2) / 0.8
    chA = [["F", 0, 0, q0, qn, cap, "V"] for (q0, qn, cap) in chunksA]
    if chA:
        # lead chunks and drain chunk direct: no GpSimd handoff at the
        # pipeline boundaries where latency, not throughput, dominates
        chA[0][0] = "-"
        if len(chA) > 1 and chA[1][4] * chA[1][5] <= 600:
            chA[1][0] = "-"
        chA[-1][0] = "-"
    # (tested: a GpSimd-mult fp8 chunk is latency-neutral at best; the
    # serial mult->fold chain on GpSimd cancels the Vector relief)
    offd = off16 = off8 = 0
    qoff_ds = np.zeros(Q, np.int64)
    qoff_xs = np.zeros(Q, np.int64)
    qeng = np.empty(Q, "U1")
    for c in chA:
        _, _, _, q0, qn, cap, eng = c
        c[1] = offd
        c[2] = off16 if eng == "V" else off8
        for q in range(q0, q0 + qn):
            qoff_ds[q] = offd + (q - q0) * cap
            qoff_xs[q] = c[2] + (q - q0) * cap
            qeng[q] = eng
        offd += qn * cap
        if eng == "V":
            off16 += qn * cap
        else:
            off8 += qn * cap
    sa_len, sa16, sa8 = offd, off16, off8

    # route edges: edge -> (core, lane, col) via dst's layout position
    ecore = LA.core[dst]
    elane = LA.lane[dst]
    eq = LA.q[dst]
    ew = _within_dst(dst)
    ecold = qoff_ds[eq] + ew
    ecolx = qoff_xs[eq] + ew
    eisv = qeng[eq] == "V"
    xv = np.zeros((NC, P, max(sa16, 1)), np.float16)
    xv8 = np.zeros((NC, P, max(sa8, 1)), E4NP)
    dv = np.ones((NC, P, sa_len), np.uint8)
    m = eisv
    xv[ecore[m], elane[m], ecolx[m]] = x[src[m]].astype(np.float16)
    m = ~eisv
    xv8[ecore[m], elane[m], ecolx[m]] = x[src[m]].astype(E4NP)
    dv[ecore, elane, ecold] = degp1[src].astype(np.uint8)

    dgp_g = LA.grid_scatter(degp1.astype(np.uint8), 1, np.uint8)
    x_g = LA.grid_scatter(x, 0.0, np.float32)

    ncA = build_kA(chA, sa16, sa8, sa_len)
    rA = run(ncA, [dict(xs=xv[k], xs8=xv8[k], ds=dv[k], degp1=dgp_g[k],
                        xg=x_g[k]) for k in range(NC)], "kA")
    y_g = np.stack([rA[k]["yg"] for k in range(NC)])
    dinv_g = np.stack([rA[k]["dinv"] for k in range(NC)])

    # ---- pass B layout & routing ----
    y_node = LA.grid_gather(y_g)          # per-node y (fp32, device-computed)
    dinv_node = LA.grid_gather(dinv_g)    # fp16 values, permutation only
    yv = y_node[src]
    epos = yv > 0
    pos = np.zeros(N, np.int64)
    np.add.at(pos, dst, epos.astype(np.int64))
    neg = deg - pos
    LB = Layout(np.maximum(pos, neg), align=4, penalty=200)
    chunksB = _split_chunks(LB.regions, target=1400, first_small=True)
    chB = []
    off = 0
    qoffB = np.zeros(Q, np.int64)
    capB = np.zeros(Q, np.int64)
    for i, (q0, qn, cap) in enumerate(chunksB):
        fold = "F" if (cap % 2 == 0 and i > 1) else "-"
        chB.append([fold, off, q0, qn, cap])
        for q in range(q0, q0 + qn):
            qoffB[q] = off + (q - q0) * 2 * cap
            capB[q] = cap
        off += qn * 2 * cap
    sb_len = off

    ecoreB = LB.core[dst]
    elaneB = LB.lane[dst]
    eqB = LB.q[dst]
    ewB = _within_dst(dst, sub=(~epos).astype(np.int64))
    ecolB = qoffB[eqB] + np.where(epos, 0, capB[eqB]) + ewB
    ysv = np.zeros((NC, P, sb_len), E4NP)
    ysv[ecoreB, elaneB, ecolB] = yv.astype(E4NP)

    dinvB_g = LB.grid_scatter(dinv_node, 0, np.float16)
    ygB_g = LB.grid_scatter(y_node, 0.0, np.float32)
    plan = _w_plan(a_vec, b_vec, b2)

    ncB = build_kB(chB, sb_len, a_vec, b_vec, b2)
    in_maps = [dict(ys=ysv[k], dinvg=dinvB_g[k], yg=ygB_g[k])
               for k in range(NC)]
    rB = run(ncB, in_maps, "k5")
    acc = np.stack([rB[k]["acc"] for k in range(NC)])

    # O(1) classifier head on host (fp64): unpermute plan order, pad-node
    # relu(b2) correction, constant features
    accp = acc.sum(axis=(0, 1)).astype(np.float64)
    pooled = np.zeros(16)
    for j, (amain, r, sig, sf, bias, f) in enumerate(plan):
        if sf == 0.0:
            pooled[f] = N * max(float(b2[f]), 0.0)
        else:
            pooled[f] = sf * accp[j] - (NPAD - N) * max(float(b2[f]), 0.0)
    pooled /= float(N)
    logits = pooled @ Wl + bl
    mm = logits.max()
    out = (logits - mm) - np.log(np.exp(logits - mm).sum())
    return out[None, :].astype(np.float32), phase_ns


def kernel(**inputs) -> np.ndarray:
    out, _ = run_pipeline(inputs, trace=False)
    return out
